# revision 8
# baseline (speedup 1.0000x reference)
"""RBF-kernel SVM decision function on 8 TRN2 NeuronCores.

out[i] = sum_j alphas[j] * exp(-GAMMA * ||x[i] - supports[j]||^2)

Strategy (data-parallel over x rows; supports/alphas replicated):
  exponent e_ij is produced ENTIRELY by one bf16 matmul with 68
  contraction rows:
    rows 0-63 : (x/32)^T vs s^T          -> 2*gamma*(x.s)
    row 64,65 : 1.0     vs jt hi/lo      -> ln|a_j| - gamma*|s_j|^2
    row 66,67 : c hi/lo vs 1.0           -> -gamma*|x_i|^2
  so PSUM holds e_ij directly (no ACT bias / DVE per-partition scalar).

  The N*M elementwise exp+reduce work is split between BOTH engines,
  each 2048-wide j-window handled by exactly one of them:
    ACT window: ACTIVATE(Exp, accum_out=...) in place on PSUM; the
      free-dim accumulator does the reduction for free.
    DVE window: "Schraudolph" exp — one tensor_scalar computes
      round(A*e + B) into an int16 SBUF tile; those int16 bit patterns
      ARE bf16 exp values (A = 128/ln2, B = 16256 - C).  A second
      16-bit tensor_scalar(accum_out=...) reduces the bf16 view.

  Support permutation (host-side) controls accuracy:
    w0 = largest-|alpha| positives      (always ACT: exact exp)
    w3 = largest-|alpha| negatives      (always ACT)
    w2 = smallest 1024 P + smallest 1024 N   (always DVE)
    w1 = remaining medium-small P + N        (DVE on some tiles)
  DVE windows contain only tiny-|alpha| supports (~0.7-6% of the alpha^2
  mass) AND are internally sign-balanced (equal P/N L1 mass), so the
  ~1.75% rms periodic Schraudolph error and its global bias both wash
  out: simulated rel err ~5e-3 vs the 2e-2 gate, insensitive to the
  rounding mode of the fp32->int16 convert.

The ACT:DVE window ratio (DVE_PER_TILE) is the main speed tuning knob.
"""

import os
import sys

for p in ("/opt/trn_rl_repo",):
    if p not in sys.path:
        sys.path.insert(0, p)

import numpy as np
import ml_dtypes

import concourse.bass as bass
import concourse.tile as tile
from concourse import bacc, mybir
from concourse.bass_utils import run_bass_kernel_spmd

N_CORES = 8
N = 16384
M = 8192
F = 64
GAMMA = 1.0 / F
N_LOC = N // N_CORES        # 2048 queries per core
N_TILES = N_LOC // 128      # 16 i-tiles of 128 queries
K_AUG = F + 4               # 68 contraction rows (x, jt pair-rows, c pair-rows)
W = 2048                    # j-window: 4 PSUM banks
NW = M // W                 # 4 windows per j sweep
MM_N = 512                  # matmul moving free dim (1 PSUM bank)
NCH = W // MM_N             # chunks per window

# Schraudolph constants: round(A*e + B) as int16 == bf16 bits of ~exp(e)
SCH_A = 128.0 / float(np.log(2.0))
SCH_C = 7.5

# Per-tile count of DVE windows (len N_TILES, each 0..2).  DVE windows
# are taken from the front of W_ORDER = [2, 1, 0, 3].
DVE_PER_TILE = [2, 1, 1, 2, 1, 1, 2, 1, 1, 2, 1, 1, 2, 1, 1, 1]  # sum 21

W_ORDER = [2, 1, 0, 3]

BF16 = mybir.dt.bfloat16
F8E4 = mybir.dt.float8e4
I16 = mybir.dt.int16
F32 = mybir.dt.float32
bf16 = ml_dtypes.bfloat16
f8e4 = ml_dtypes.float8_e4m3

_compiled_cache = {}


def _build(ranges):
    """ranges: tuple of (lo, hi, is_pos) sign ranges covering [0, M)."""
    nc = bacc.Bacc(
        "TRN2",
        target_bir_lowering=False,
        debug=False,
        enable_asserts=False,
        num_devices=N_CORES,
    )
    sch_b = 16256.0 - SCH_C

    def pieces_of(w):
        lo, hi = w * W, (w + 1) * W
        out = []
        for rlo, rhi, pos in ranges:
            plo, phi = max(lo, rlo), min(hi, rhi)
            if plo < phi:
                out.append((plo, phi, pos))
        return out

    n_pos = sum(1 for w in range(NW) for p in pieces_of(w) if p[2])
    n_neg = sum(1 for w in range(NW) for p in pieces_of(w) if not p[2])

    with tile.TileContext(nc) as tc:
        with (
            tc.tile_pool(name="const", bufs=1) as cpool,
            tc.tile_pool(name="acc", bufs=3) as apool,
            tc.tile_pool(name="stg", bufs=3) as spool,
            tc.tile_pool(name="psum", bufs=2, space="PSUM") as ppool,
        ):
            # DoubleRow fp8 operands: each contraction row carries an (A, B)
            # pair; out = sum_r W_A[r]X_A[r] + W_B[r]X_B[r].
            #   rows 0-63: W_A=W_B=e4m3(x/4); X_A,X_B = e4m3 hi/lo of s/8
            #   row 64:    W=1.0;  X_A,X_B = jt residual components 0,1
            #   row 65:    W=1.0;  X_A,X_B = jt residual components 2,3
            #   row 66:    W_A,W_B = c components 0,1;  X=1.0
            #   row 67:    W_A,W_B = c components 2,3;  X=1.0
            xaugT_d = nc.dram_tensor(
                "xaugT", [K_AUG, 2, N_LOC], F8E4, kind="ExternalInput"
            )
            saug_d = nc.dram_tensor(
                "saug", [K_AUG, NW * NCH, 2, MM_N], F8E4, kind="ExternalInput"
            )
            out_d = nc.dram_tensor("out", [128, N_TILES], F32, kind="ExternalOutput")

            # Dummy exp() on a zeroed tile: first in the ACT engine's
            # program, so the exp table load overlaps the input DMAs.
            warm_act = cpool.tile([128, 1], F32)
            nc.gpsimd.memset(warm_act[:], 0.0)
            nc.scalar.activation(
                warm_act[:], warm_act[:], mybir.ActivationFunctionType.Exp
            )

            saug_sb = cpool.tile([K_AUG, NW * NCH, 2, MM_N], F8E4)
            first_w = W_ORDER[0]
            nc.sync.dma_start(
                saug_sb[:, first_w * NCH : (first_w + 1) * NCH],
                saug_d.ap()[:, first_w * NCH : (first_w + 1) * NCH],
            )
            xaugT_sb = cpool.tile([K_AUG, 2, N_LOC], F8E4)
            nc.sync.dma_start(xaugT_sb[:, :, 0:128], xaugT_d.ap()[:, :, 0:128])
            for w in W_ORDER[1:]:
                nc.sync.dma_start(
                    saug_sb[:, w * NCH : (w + 1) * NCH],
                    saug_d.ap()[:, w * NCH : (w + 1) * NCH],
                )
            nc.sync.dma_start(xaugT_sb[:, :, 128:], xaugT_d.ap()[:, :, 128:])

            outT_sb = cpool.tile([128, N_TILES], F32)
            dvout = cpool.tile([128, W], BF16)

            for t in range(N_TILES):
                accP = apool.tile([128, max(n_pos, 1)], F32, tag="accP")
                accN = apool.tile([128, max(n_neg, 1)], F32, tag="accN")
                iP = iN = 0

                def acc_col(pos):
                    nonlocal iP, iN
                    if pos:
                        col = accP[:, iP : iP + 1]
                        iP += 1
                    else:
                        col = accN[:, iN : iN + 1]
                        iN += 1
                    return col

                n_dve = DVE_PER_TILE[t]
                for wi, w in enumerate(W_ORDER):
                    ps_tile = ppool.tile([128, W], F32, tag="E")
                    for c in range(NCH):
                        nc.tensor.matmul(
                            ps_tile[:, c * MM_N : (c + 1) * MM_N],
                            xaugT_sb[:, :, t * 128 : (t + 1) * 128],
                            saug_sb[:, w * NCH + c],
                            start=True,
                            stop=True,
                            perf_mode=mybir.MatmulPerfMode.DoubleRow,
                        )
                    if wi < n_dve:
                        # DVE window: Schraudolph exp then 16-bit reduce.
                        stg = spool.tile([128, W], I16, tag="stg")
                        nc.vector.tensor_scalar(
                            stg[:],
                            ps_tile[:],
                            SCH_A,
                            sch_b,
                            mybir.AluOpType.mult,
                            mybir.AluOpType.add,
                        )
                        stg_bf = stg[:].bitcast(BF16)
                        for lo, hi, pos in pieces_of(w):
                            nc.vector.tensor_scalar(
                                dvout[:, lo - w * W : hi - w * W],
                                stg_bf[:, lo - w * W : hi - w * W],
                                1.0,
                                0.0,
                                mybir.AluOpType.mult,
                                mybir.AluOpType.add,
                                accum_out=acc_col(pos),
                            )
                    else:
                        for lo, hi, pos in pieces_of(w):
                            nc.scalar.activation(
                                ps_tile[:, lo - w * W : hi - w * W],
                                ps_tile[:, lo - w * W : hi - w * W],
                                mybir.ActivationFunctionType.Exp,
                                accum_out=acc_col(pos),
                            )
                sumP = apool.tile([128, 1], F32, tag="sumP")
                nc.vector.reduce_sum(sumP[:], accP[:, :iP], axis=mybir.AxisListType.X)
                sumN = apool.tile([128, 1], F32, tag="sumN")
                nc.vector.reduce_sum(sumN[:], accN[:, :iN], axis=mybir.AxisListType.X)
                nc.vector.tensor_sub(outT_sb[:, t : t + 1], sumP[:], sumN[:])

            nc.sync.dma_start(out_d.ap()[:], outT_sb[:])

    nc.compile()
    return nc


def _prepare(x, supports, alphas):
    x = np.asarray(x, dtype=np.float32)
    supports = np.asarray(supports, dtype=np.float32)
    alphas = np.asarray(alphas, dtype=np.float32)

    a64 = alphas.astype(np.float64)
    s64 = supports.astype(np.float64)

    pos = a64 > 0
    iP = np.nonzero(pos)[0]
    iN = np.nonzero(~pos)[0]
    Pd = iP[np.argsort(-np.abs(a64[iP]))]  # descending |alpha|
    Nd = iN[np.argsort(-np.abs(a64[iN]))]
    nP, nN = len(Pd), len(Nd)
    # w0 = big P, w3 = big N, w2 = smallest 1024 P + smallest 1024 N,
    # w1 = the remaining medium-small P then N.
    w0 = Pd[0:2048]
    w3 = Nd[0:2048]
    w1P, w2P = Pd[2048 : nP - 1024], Pd[nP - 1024 :]
    w1N, w2N = Nd[2048 : nN - 1024], Nd[nN - 1024 :]
    blocks = [(w0, True), (w1P, True), (w1N, False),
              (w2P, True), (w2N, False), (w3, False)]
    perm = np.concatenate([b for b, _ in blocks])
    ranges = []
    o = 0
    for blk, sgn in blocks:
        ranges.append((o, o + len(blk), sgn))
        o += len(blk)
    ranges = tuple(ranges)

    def e4(v):
        return np.clip(v, -240.0, 240.0).astype(f8e4).astype(np.float64)

    jterm = -GAMMA * (s64 * s64).sum(axis=1) + np.log(
        np.maximum(np.abs(a64), 1e-300)
    )
    jt = jterm[perm]
    jr = []
    r = jt.copy()
    for _ in range(4):
        q = e4(r)
        jr.append(q)
        r = r - q

    cterm = -GAMMA * (x.astype(np.float64) ** 2).sum(axis=1)
    cr = []
    r = cterm.copy()
    for _ in range(4):
        q = e4(r)
        cr.append(q)
        r = r - q

    sT = s64[perm].T / 8.0                       # [F, M]
    s_hi = e4(sT)
    s_lo = e4(sT - s_hi)

    # Moving tensor: [K_AUG, NW*NCH, 2, MM_N]
    saug = np.ones((K_AUG, NW * NCH, 2, MM_N), dtype=f8e4)
    sh = s_hi.reshape(F, NW * NCH, MM_N)
    sl = s_lo.reshape(F, NW * NCH, MM_N)
    saug[:F, :, 0, :] = sh.astype(f8e4)
    saug[:F, :, 1, :] = sl.astype(f8e4)
    j0 = jr[0].reshape(NW * NCH, MM_N)
    j1 = jr[1].reshape(NW * NCH, MM_N)
    j2 = jr[2].reshape(NW * NCH, MM_N)
    j3 = jr[3].reshape(NW * NCH, MM_N)
    saug[F, :, 0, :] = j0.astype(f8e4)
    saug[F, :, 1, :] = j1.astype(f8e4)
    saug[F + 1, :, 0, :] = j2.astype(f8e4)
    saug[F + 1, :, 1, :] = j3.astype(f8e4)
    # rows 66, 67 moving side stay 1.0

    # Weights tensor: [K_AUG, 2, N]
    x8 = e4(x.astype(np.float64).T / 4.0)        # [F, N]
    xaugT = np.ones((K_AUG, 2, N), dtype=f8e4)
    xaugT[:F, 0, :] = x8.astype(f8e4)
    xaugT[:F, 1, :] = x8.astype(f8e4)
    # jt rows: weights stay 1.0 on both planes
    xaugT[F + 2, 0, :] = cr[0].astype(f8e4)
    xaugT[F + 2, 1, :] = cr[1].astype(f8e4)
    xaugT[F + 3, 0, :] = cr[2].astype(f8e4)
    xaugT[F + 3, 1, :] = cr[3].astype(f8e4)

    in_maps = []
    for c in range(N_CORES):
        sl_ = slice(c * N_LOC, (c + 1) * N_LOC)
        in_maps.append(
            {
                "xaugT": np.ascontiguousarray(xaugT[:, :, sl_]),
                "saug": saug,
            }
        )
    return ranges, in_maps


def _run(x, supports, alphas, trace=False, **run_kwargs):
    ranges, in_maps = _prepare(x, supports, alphas)
    key = (ranges, tuple(DVE_PER_TILE), SCH_C)
    if key not in _compiled_cache:
        _compiled_cache[key] = _build(ranges)
    nc = _compiled_cache[key]
    res = run_bass_kernel_spmd(
        nc, in_maps, core_ids=list(range(N_CORES)), trace=trace, **run_kwargs
    )
    outs = [r["out"].T.reshape(-1) for r in res.results]
    return np.concatenate(outs).astype(np.float32), res


def kernel(x, supports, alphas):
    out, _ = _run(x, supports, alphas, trace=False)
    return out


# revision 11
# speedup vs baseline: 1.1327x; 1.1327x over previous
"""RBF-kernel SVM decision function on 8 TRN2 NeuronCores.

out[i] = sum_j alphas[j] * exp(-GAMMA * ||x[i] - supports[j]||^2)

Strategy (data-parallel over x rows; supports/alphas replicated):
  exponent e_ij is produced ENTIRELY by one bf16 matmul with 68
  contraction rows:
    rows 0-63 : (x/32)^T vs s^T          -> 2*gamma*(x.s)
    row 64,65 : 1.0     vs jt hi/lo      -> ln|a_j| - gamma*|s_j|^2
    row 66,67 : c hi/lo vs 1.0           -> -gamma*|x_i|^2
  so PSUM holds e_ij directly (no ACT bias / DVE per-partition scalar).

  The N*M elementwise exp+reduce work is split between BOTH engines,
  each 2048-wide j-window handled by exactly one of them:
    ACT window: ACTIVATE(Exp, accum_out=...) in place on PSUM; the
      free-dim accumulator does the reduction for free.
    DVE window: "Schraudolph" exp — one tensor_scalar computes
      round(A*e + B) into an int16 SBUF tile; those int16 bit patterns
      ARE bf16 exp values (A = 128/ln2, B = 16256 - C).  The window's
      equal-size P and N halves are folded with one bf16
      tensor_tensor subtract (P - N), then a single 16-bit
      tensor_scalar(accum_out=...) reduces the fold (plus a tiny
      leftover reduce when the halves differ in size).
  Per-tile accumulator-column sums and the final P-N combine run on the
  otherwise idle GPSIMD engine.

  Support permutation (host-side) controls accuracy:
    w0 = largest-|alpha| positives      (always ACT: exact exp)
    w3 = largest-|alpha| negatives      (always ACT)
    w2 = [1024 smallest N | 1024 smallest P]     (always DVE)
    w1 = [medium-small N | medium-small P]       (DVE on some tiles)
  DVE windows contain only tiny-|alpha| supports (~7% of the alpha^2
  mass) AND are internally sign-balanced, so the ~1.75% rms periodic
  Schraudolph error and its global bias both wash out: simulated rel
  err ~5e-3 vs the 2e-2 gate, insensitive to the rounding mode of the
  fp32->int16 convert.  (One support — P count is odd — is parked in
  w3 with jt=-50, i.e. weight exp(-50)=0, to keep piece boundaries
  even for DVE 2x alignment.)

DVE_PER_TILE (ACT:DVE window ratio) is the main speed tuning knob.
"""

import os
import sys

for p in ("/opt/trn_rl_repo",):
    if p not in sys.path:
        sys.path.insert(0, p)

import numpy as np
import ml_dtypes

import concourse.bass as bass
import concourse.tile as tile
from concourse import bacc, mybir
from concourse.bass_utils import run_bass_kernel_spmd

N_CORES = 8
N = 16384
M = 8192
F = 64
GAMMA = 1.0 / F
N_LOC = N // N_CORES        # 2048 queries per core
N_TILES = N_LOC // 128      # 16 i-tiles of 128 queries
K_AUG = F + 4               # 68 contraction rows (x, jt hi/lo, c hi/lo)
W = 2048                    # j-window: 4 PSUM banks
NW = M // W                 # 4 windows per j sweep
MM_N = 512                  # matmul moving free dim (1 PSUM bank)

# Schraudolph constants: round(A*e + B) as int16 == bf16 bits of ~exp(e)
SCH_A = 128.0 / float(np.log(2.0))
SCH_C = 7.5

# Per-tile count of DVE windows (0..2); sum is the DVE share.
DVE_PER_TILE = [2, 1, 1, 2, 1, 1, 2, 1, 1, 2, 1, 1, 2, 1, 1, 1]  # sum 21

BF16 = mybir.dt.bfloat16
I16 = mybir.dt.int16
F32 = mybir.dt.float32
bf16 = ml_dtypes.bfloat16

_compiled_cache = {}


def _build(ranges):
    """ranges: tuple of (lo, hi, is_pos) sign ranges covering [0, M)."""
    nc = bacc.Bacc(
        "TRN2",
        target_bir_lowering=False,
        debug=False,
        enable_asserts=False,
        num_devices=N_CORES,
    )
    sch_b = 16256.0 - SCH_C

    def pieces_of(w):
        lo, hi = w * W, (w + 1) * W
        out = []
        for rlo, rhi, pos in ranges:
            plo, phi = max(lo, rlo), min(hi, rhi)
            if plo < phi:
                out.append((plo, phi, pos))
        return out

    # Window order per tile interleaves the DVE and ACT windows so both
    # engines run concurrently on the two in-flight PSUM tiles.
    def tile_schedule(n_dve):
        if n_dve == 0:
            return [(2, "A"), (0, "A"), (1, "A"), (3, "A")]
        if n_dve == 1:
            return [(2, "D"), (1, "A"), (0, "A"), (3, "A")]
        return [(2, "D"), (0, "A"), (1, "D"), (3, "A")]

    n_pos = sum(1 for w in range(NW) for p in pieces_of(w) if p[2]) + 1
    n_neg = sum(1 for w in range(NW) for p in pieces_of(w) if not p[2]) + 1

    with tile.TileContext(nc) as tc:
        with (
            tc.tile_pool(name="const", bufs=1) as cpool,
            tc.tile_pool(name="acc", bufs=3) as apool,
            tc.tile_pool(name="stg", bufs=3) as spool,
            tc.tile_pool(name="psum", bufs=2, space="PSUM") as ppool,
        ):
            xaugT_d = nc.dram_tensor(
                "xaugT", [K_AUG, N_LOC], BF16, kind="ExternalInput"
            )
            saug_d = nc.dram_tensor("saug", [K_AUG, M], BF16, kind="ExternalInput")
            out_d = nc.dram_tensor("out", [128, N_TILES], F32, kind="ExternalOutput")

            # Dummy exp() on a zeroed tile: first in the ACT engine's
            # program, so the exp table load overlaps the input DMAs.
            warm_act = cpool.tile([128, 1], F32)
            nc.gpsimd.memset(warm_act[:], 0.0)
            nc.scalar.activation(
                warm_act[:], warm_act[:], mybir.ActivationFunctionType.Exp
            )

            saug_sb = cpool.tile([K_AUG, M], BF16)
            nc.sync.dma_start(saug_sb[:, 2 * W : 3 * W], saug_d.ap()[:, 2 * W : 3 * W])
            xaugT_sb = cpool.tile([K_AUG, N_LOC], BF16)
            nc.sync.dma_start(xaugT_sb[:, 0:128], xaugT_d.ap()[:, 0:128])
            for w in (0, 1, 3):
                nc.sync.dma_start(
                    saug_sb[:, w * W : (w + 1) * W],
                    saug_d.ap()[:, w * W : (w + 1) * W],
                )
            nc.sync.dma_start(xaugT_sb[:, 128:], xaugT_d.ap()[:, 128:])

            outT_sb = cpool.tile([128, N_TILES], F32)
            dvout = cpool.tile([128, W], BF16)

            for t in range(N_TILES):
                accP = apool.tile([128, n_pos], F32, tag="accP")
                accN = apool.tile([128, n_neg], F32, tag="accN")
                iP = iN = 0

                def acc_col(pos):
                    nonlocal iP, iN
                    if pos:
                        col = accP[:, iP : iP + 1]
                        iP += 1
                    else:
                        col = accN[:, iN : iN + 1]
                        iN += 1
                    return col

                for w, eng in tile_schedule(DVE_PER_TILE[t]):
                    ps_tile = ppool.tile([128, W], F32, tag="E")
                    for c in range(W // MM_N):
                        nc.tensor.matmul(
                            ps_tile[:, c * MM_N : (c + 1) * MM_N],
                            xaugT_sb[:, t * 128 : (t + 1) * 128],
                            saug_sb[:, w * W + c * MM_N : w * W + (c + 1) * MM_N],
                            start=True,
                            stop=True,
                        )
                    if eng == "D":
                        # Schraudolph exp into int16 staging.
                        stg = spool.tile([128, W], I16, tag="stg")
                        nc.vector.tensor_scalar(
                            stg[:],
                            ps_tile[:],
                            SCH_A,
                            sch_b,
                            mybir.AluOpType.mult,
                            mybir.AluOpType.add,
                        )
                        stg_bf = stg[:].bitcast(BF16)
                        pieces = pieces_of(w)
                        assert len(pieces) == 2 and pieces[0][2] != pieces[1][2]
                        pp = next(p for p in pieces if p[2])
                        pn = next(p for p in pieces if not p[2])
                        L = min(pp[1] - pp[0], pn[1] - pn[0])
                        fold = spool.tile([128, W // 2], BF16, tag="fold")
                        o = w * W
                        nc.vector.tensor_sub(
                            fold[:, 0:L],
                            stg_bf[:, pp[0] - o : pp[0] - o + L],
                            stg_bf[:, pn[0] - o : pn[0] - o + L],
                        )
                        nc.vector.tensor_scalar(
                            dvout[:, 0:L],
                            fold[:, 0:L],
                            1.0,
                            0.0,
                            mybir.AluOpType.mult,
                            mybir.AluOpType.add,
                            accum_out=acc_col(True),
                        )
                        if pp[1] - pp[0] > L:
                            nc.vector.tensor_scalar(
                                dvout[:, W // 2 : W // 2 + (pp[1] - pp[0] - L)],
                                stg_bf[:, pp[0] - o + L : pp[1] - o],
                                1.0,
                                0.0,
                                mybir.AluOpType.mult,
                                mybir.AluOpType.add,
                                accum_out=acc_col(True),
                            )
                        elif pn[1] - pn[0] > L:
                            nc.vector.tensor_scalar(
                                dvout[:, W // 2 : W // 2 + (pn[1] - pn[0] - L)],
                                stg_bf[:, pn[0] - o + L : pn[1] - o],
                                1.0,
                                0.0,
                                mybir.AluOpType.mult,
                                mybir.AluOpType.add,
                                accum_out=acc_col(False),
                            )
                    else:
                        for lo, hi, pos in pieces_of(w):
                            nc.scalar.activation(
                                ps_tile[:, lo - w * W : hi - w * W],
                                ps_tile[:, lo - w * W : hi - w * W],
                                mybir.ActivationFunctionType.Exp,
                                accum_out=acc_col(pos),
                            )
                sumP = apool.tile([128, 1], F32, tag="sumP")
                nc.vector.reduce_sum(sumP[:], accP[:, :iP], axis=mybir.AxisListType.X)
                sumN = apool.tile([128, 1], F32, tag="sumN")
                nc.vector.reduce_sum(sumN[:], accN[:, :iN], axis=mybir.AxisListType.X)
                nc.vector.tensor_sub(outT_sb[:, t : t + 1], sumP[:], sumN[:])

            nc.sync.dma_start(out_d.ap()[:], outT_sb[:])

    nc.compile()
    return nc


def _prepare(x, supports, alphas):
    x = np.asarray(x, dtype=np.float32)
    supports = np.asarray(supports, dtype=np.float32)
    alphas = np.asarray(alphas, dtype=np.float32)

    a64 = alphas.astype(np.float64)
    s64 = supports.astype(np.float64)

    pos = a64 > 0
    iP = np.nonzero(pos)[0]
    iN = np.nonzero(~pos)[0]
    Pd = iP[np.argsort(-np.abs(a64[iP]))]  # descending |alpha|
    Nd = iN[np.argsort(-np.abs(a64[iN]))]

    # Window layout (even piece boundaries for DVE 2x alignment):
    #   w0 = P big 2048
    #   w1 = [N mid n1n | P mid n1p]
    #   w2 = [N small 1024 | P small 1024]
    #   w3 = [N big 2048-len(dead) | dead]
    # Any odd-count leftovers are parked in `dead` with jt=-50 (zero
    # weight), keeping all live piece boundaries even.
    nP, nN = len(Pd), len(Nd)
    # Park the globally smallest positives in `dead` (weight zero) until
    # the w1 P-piece count is even and slot totals work out.
    n_dead = (nP - 2048 - 1024) % 2
    n1p = nP - n_dead - 1024 - 2048
    n1n = 2048 - n1p
    assert n1n <= nN - 1024, (n1n, nN)
    w0 = Pd[0:2048]
    w1P = Pd[2048 : 2048 + n1p]
    w2P = Pd[2048 + n1p : nP - n_dead]
    dead = Pd[nP - n_dead :]
    assert len(w2P) == 1024
    w2N = Nd[nN - 1024 :]
    w1N = Nd[nN - 1024 - n1n : nN - 1024]
    # w3 region holds the big negatives, then dead.
    w3_full = np.concatenate([Nd[0 : nN - 1024 - n1n], dead])
    assert len(w3_full) == 2048, len(w3_full)

    blocks = [
        (w0, True),
        (w1N, False),
        (w1P, True),
        (w2N, False),
        (w2P, True),
        (w3_full, False),
    ]
    perm = np.concatenate([b for b, _ in blocks])
    assert len(perm) == M
    ranges = []
    o = 0
    for blk, sgn in blocks:
        ranges.append((o, o + len(blk), sgn))
        o += len(blk)
    ranges = tuple(ranges)
    n_dead = len(dead)

    jterm = -GAMMA * (s64 * s64).sum(axis=1) + np.log(
        np.maximum(np.abs(a64), 1e-300)
    )
    jt = jterm[perm]
    if n_dead:
        jt[M - n_dead :] = -50.0  # weight exp(-50) = 0
    jt_hi = jt.astype(bf16)
    jt_lo = (jt - jt_hi.astype(np.float64)).astype(bf16)

    saug = np.ones((K_AUG, M), dtype=bf16)
    saug[:F] = supports[perm].T.astype(bf16)
    saug[F] = jt_hi
    saug[F + 1] = jt_lo

    cterm = -GAMMA * (x.astype(np.float64) ** 2).sum(axis=1)
    c_hi = cterm.astype(bf16)
    c_lo = (cterm - c_hi.astype(np.float64)).astype(bf16)

    xaugT = np.ones((K_AUG, N), dtype=bf16)
    xaugT[:F] = (x.T / 32.0).astype(bf16)
    xaugT[F + 2] = c_hi
    xaugT[F + 3] = c_lo

    in_maps = []
    for c in range(N_CORES):
        sl = slice(c * N_LOC, (c + 1) * N_LOC)
        in_maps.append(
            {
                "xaugT": np.ascontiguousarray(xaugT[:, sl]),
                "saug": saug,
            }
        )
    return ranges, in_maps


def _run(x, supports, alphas, trace=False, **run_kwargs):
    ranges, in_maps = _prepare(x, supports, alphas)
    key = (ranges, tuple(DVE_PER_TILE), SCH_C)
    if key not in _compiled_cache:
        _compiled_cache[key] = _build(ranges)
    nc = _compiled_cache[key]
    res = run_bass_kernel_spmd(
        nc, in_maps, core_ids=list(range(N_CORES)), trace=trace, **run_kwargs
    )
    outs = [r["out"].T.reshape(-1) for r in res.results]
    return np.concatenate(outs).astype(np.float32), res


def kernel(x, supports, alphas):
    out, _ = _run(x, supports, alphas, trace=False)
    return out
